# revision 13
# baseline (speedup 1.0000x reference)
"""Trainium2 Bass kernel for nn_EquivariantTransformerBlock.

Strategy (8 NeuronCores, no collectives):
  - Host assigns each node to one of 320 "buckets" of 128 nodes (degree-
    balanced snake packing).  Core c owns buckets [40c, 40c+40); every edge
    goes to the core owning its dst bucket, so all segment sums are local.
  - Per-edge compute uses an edges-on-partitions layout (128 edges/chunk):
      * edge MLP on the TensorE with the per-chunk h as the stationary
        operand (gives rw in (edge, feature) layout directly),
      * the per-edge bilinear contractions (tmp = fU@basis, conv = rw@tmp)
        as broadcast-view products + halving-tree sums on the VectorE,
      * softmax without max-subtraction (scores are O(40), exp is safe;
        softmax is shift-invariant so this matches the reference
        numerically),
      * segment sums as one-hot matmuls accumulated in PSUM per bucket.
  - The equivariant LayerNorm runs replicated on every core; its output f
    is written to DRAM and gathered per-edge with indirect DMA.
"""

import math
from contextlib import ExitStack
from dataclasses import dataclass

import numpy as np

# ---------------------------------------------------------------------------
# Problem constants (hardcoded; kernel.py must be self-contained)
# ---------------------------------------------------------------------------
N_NODES = 40000
N_EDGES = 320000
EDGE_DIM = 32
EDGE_HIDDEN = 64
M1, D1 = 8, 4
M2, D2 = 8, 4
NREPS1 = 2
NREPS_PROD = 4
N_HEADS = 4
TMP_OUT = 768            # 24 x 32
LN_EPS = 1e-5
EQ_EPS = 1e-8
IX1 = np.array([0, 1, 1, 1])
IX2 = np.array([0, 1, 1, 1])

N_CORES = 8
BUCKET_N = 128           # nodes per bucket == one-hot window == PSUM tile
NB = 40                  # buckets per core
NODES_PAD = N_CORES * NB * BUCKET_N   # 40960
SCALE = 1.0 / math.sqrt(M2 * D2 * 1.0 * 32 / 32)  # placeholder, set below
SCALE = (M2 * D2) ** -0.5 / math.sqrt(32 / (M2 * D2))  # = 32**-0.5
SCALE = 32.0 ** -0.5
OPT = {"bf16": False, "split": 0}


@dataclass
class Cfg:
    nb: int              # buckets per core
    kb: int              # chunks per bucket
    ln_chunks: int       # node chunks (nodes_tbl/128) for LN
    ln_pass: int         # LN chunks per pass
    nodes_tbl: int       # rows in the f table (global padded nodes)
    slots: int = 32      # per-node edge slots (pow2 >= max degree)

    @property
    def ch(self):
        return self.nb * self.kb

    @property
    def e_pad(self):
        return self.ch * 128


# ---------------------------------------------------------------------------
# Patches: this walrus build allows at most ONE sync wait per instruction.
# ---------------------------------------------------------------------------
_PATCHED = False


def _apply_patches():
    global _PATCHED
    if _PATCHED:
        return
    _PATCHED = True
    import re as _re

    import orjson as _orjson

    import concourse.bass as _bass
    from concourse.tile import TileContext as _TC
    from concourse.vector_clock import ScopedClock as _SC, VectorClock as _VC

    def _drain_and_barrier(self, tick_clock, wait_clock):
        nc = self.nc
        gvals = [int(x) for x in _re.findall(r"\d+", repr(tick_clock.global_clock))]
        nz = [(p, v) for p, v in enumerate(gvals) if v > 0]
        if not nz:
            nc.sync.drain()
        for p, v in nz:
            pvc = _VC()
            pvc.require_at_least(p, v)
            d = nc.sync.drain()
            wait_clock.add_sem_waits(d.ins, _SC({None: pvc}))
        nc.all_engine_barrier()
        assert self.sems is not None
        popped = nc._tile_sem_poison_stack.pop()
        assert popped is self._sem_poison
        nc.clear_and_free_semaphores(list(self.sems.allocated().values()))
        nc.all_engine_barrier()

    def _split_multi_waits(data: bytes) -> bytes:
        j = _orjson.loads(data)
        for fn in j.get("functions", []):
            for bb in fn.get("blocks", []):
                out = []
                for ins in bb.get("instructions", []):
                    si = ins.get("sync_info")
                    ow = (si or {}).get("on_wait") or []
                    if len(ow) > 1:
                        for k, w in enumerate(ow[:-1]):
                            out.append({
                                "debug": ins.get("debug", 0),
                                "engine": ins["engine"],
                                "ins": [],
                                "name": f"{ins['name']}-spw{k}",
                                "opcode": "EventSemaphore",
                                "outs": [],
                                "sync_info": {"on_update": [], "on_wait": [w]},
                            })
                        si["on_wait"] = [ow[-1]]
                    out.append(ins)
                bb["instructions"] = out
        return _orjson.dumps(j)

    _orig_to_json_bytes = _bass.Bass.to_json_bytes

    def _to_json_bytes(self):
        return _split_multi_waits(_orig_to_json_bytes(self))

    _TC._drain_and_barrier = _drain_and_barrier
    _bass.Bass.to_json_bytes = _to_json_bytes


# ---------------------------------------------------------------------------
# Device kernel builder
# ---------------------------------------------------------------------------
def build_kernel(nc, cfg: Cfg):
    import concourse.bass as bass
    import concourse.mybir as mybir
    from concourse.bass import IndirectOffsetOnAxis
    from concourse.tile import TileContext

    f32 = mybir.dt.float32
    i32 = mybir.dt.int32
    Alu = mybir.AluOpType
    Act = mybir.ActivationFunctionType

    NBk, KB, CH, E_PAD = cfg.nb, cfg.kb, cfg.ch, cfg.e_pad
    NTBL = cfg.nodes_tbl
    LNC, LNP = cfg.ln_chunks, cfg.ln_pass

    # ---- DRAM I/O -----------------------------------------------------
    basis_d = nc.dram_tensor("basis_s", (E_PAD, 64), f32, kind="ExternalInput")
    eft_d = nc.dram_tensor("eft_s", (32, E_PAD), f32, kind="ExternalInput")
    srcidx_d = nc.dram_tensor("srcidx_s", (E_PAD, 1), i32, kind="ExternalInput")
    slot_d = nc.dram_tensor("slot_s", (E_PAD, 1), i32, kind="ExternalInput")
    featcm_d = nc.dram_tensor("featcm_s", (128, LNC * 32), f32, kind="ExternalInput")
    lnw_d = nc.dram_tensor("lnw_s", (128, 16), f32, kind="ExternalInput")
    lnb_d = nc.dram_tensor("lnb_s", (128, 16), f32, kind="ExternalInput")
    w1t_d = nc.dram_tensor("w1t_s", (32, 64), f32, kind="ExternalInput")
    b1_d = nc.dram_tensor("b1_s", (64, 1), f32, kind="ExternalInput")
    w2b_d = nc.dram_tensor("w2b_s", (65, 768), f32, kind="ExternalInput")
    proj_d = nc.dram_tensor("proj_s", (128, 256), f32, kind="ExternalInput")
    out_d = nc.dram_tensor("out_s", (NBk * 128, 32), f32, kind="ExternalOutput")

    def vap(tile_ap, offset, dims):
        base = tile_ap
        return bass.AP(base.tensor, base.offset + offset, dims)

    with TileContext(nc) as tc:
        with ExitStack() as ctx:
            cpool = ctx.enter_context(tc.tile_pool(name="consts", bufs=1))
            dpool = ctx.enter_context(
                tc.tile_pool(name="dram", bufs=1, space="DRAM"))

            f_dram = dpool.tile([NTBL, 32], f32)

            w1t_t = cpool.tile([32, 64], f32)
            nc.sync.dma_start(out=w1t_t[:], in_=w1t_d.ap())
            b1_t = cpool.tile([64, 1], f32)
            nc.sync.dma_start(out=b1_t[:], in_=b1_d.ap())
            w2b_t = cpool.tile([65, 768], f32)
            nc.sync.dma_start(out=w2b_t[:], in_=w2b_d.ap())
            proj_t = cpool.tile([128, 256], f32)
            nc.sync.dma_start(out=proj_t[:], in_=proj_d.ap())

            S = cfg.slots
            RPB = 128 * S                       # scatter rows per bucket
            sc_dram = dpool.tile([NBk * RPB + 1, 36], f32)
            # prefill pattern: scl cols -1e30, v cols 0
            pref = cpool.tile([128, S * 36], f32)
            nc.vector.memset(
                vap(pref[:], 0, [[S * 36, 128], [36, S], [1, 4]]), -1e30)
            nc.vector.memset(
                vap(pref[:], 4, [[S * 36, 128], [36, S], [1, 32]]), 0.0)
            for b in range(NBk):
                nc.sync.dma_start(
                    out=vap(sc_dram[:], b * RPB * 36,
                            [[S * 36, 128], [1, S * 36]]),
                    in_=pref[:])
            outn = cpool.tile([128, NBk * 32], f32)

            # ============ Phase 1: equivariant LayerNorm ================
            with ExitStack() as lctx:
                lpool = lctx.enter_context(tc.tile_pool(name="ln", bufs=2))
                lnw_t = None
                lnw_t = cpool.tile([128, 16], f32)
                nc.sync.dma_start(out=lnw_t[:], in_=lnw_d.ap())
                lnb_t = cpool.tile([128, 16], f32)
                nc.sync.dma_start(out=lnb_t[:], in_=lnb_d.ap())

                n_pass = (LNC + LNP - 1) // LNP
                for p in range(n_pass):
                    C = min(LNP, LNC - p * LNP)
                    W = C * 32
                    feat = lpool.tile([128, LNP * 32], f32, tag="feat")
                    nc.sync.dma_start(
                        out=feat[:, 0:W],
                        in_=vap(featcm_d.ap(), p * LNP * 32,
                                [[LNC * 32, 128], [1, W]]))
                    fa = feat[:]
                    # sq = feat^2
                    sq = lpool.tile([128, LNP * 32], f32, tag="sq")
                    nc.vector.tensor_tensor(
                        sq[:, 0:W], feat[:, 0:W], feat[:, 0:W], Alu.mult)
                    sqa = sq[:]
                    # nsq (c, m, r): r0 = sq[d0]; r1 = sq[d1]+sq[d2]+sq[d3]
                    nsq = lpool.tile([128, LNP * 16], f32, tag="nsq")
                    nsqa = nsq[:]
                    nc.vector.tensor_copy(
                        vap(nsqa, 0, [[LNP * 16, 128], [16, C], [2, 8]]),
                        vap(sqa, 0, [[LNP * 32, 128], [32, C], [4, 8]]))
                    nc.vector.tensor_tensor(
                        vap(nsqa, 1, [[LNP * 16, 128], [16, C], [2, 8]]),
                        vap(sqa, 1, [[LNP * 32, 128], [32, C], [4, 8]]),
                        vap(sqa, 2, [[LNP * 32, 128], [32, C], [4, 8]]),
                        Alu.add)
                    nc.vector.tensor_tensor(
                        vap(nsqa, 1, [[LNP * 16, 128], [16, C], [2, 8]]),
                        vap(nsqa, 1, [[LNP * 16, 128], [16, C], [2, 8]]),
                        vap(sqa, 3, [[LNP * 32, 128], [32, C], [4, 8]]),
                        Alu.add)
                    # norms = sqrt(nsq)
                    norms = lpool.tile([128, LNP * 16], f32, tag="norms")
                    nrma = norms[:]
                    nc.scalar.activation(norms[:, 0:C * 16], nsq[:, 0:C * 16],
                                         Act.Sqrt)
                    # mu_raw = sum over groups of 8
                    mur = lpool.tile([128, LNP * 2], f32, tag="mur")
                    nc.vector.tensor_reduce(
                        mur[:, 0:C * 2],
                        vap(nrma, 0, [[LNP * 16, 128], [8, C * 2], [1, 8]]),
                        mybir.AxisListType.X, Alu.add)
                    # centered = norms - mu_raw/8
                    cen = lpool.tile([128, LNP * 16], f32, tag="cen")
                    cena = cen[:]
                    nc.vector.scalar_tensor_tensor(
                        cen[:, 0:C * 16],
                        vap(mur[:], 0, [[LNP * 2, 128], [1, C * 2], [0, 8]]),
                        -0.125,
                        norms[:, 0:C * 16],
                        Alu.mult, Alu.add)
                    # var_raw = sum over groups of centered^2
                    sq2 = lpool.tile([128, LNP * 16], f32, tag="sq2")
                    nc.vector.tensor_tensor(
                        sq2[:, 0:C * 16], cen[:, 0:C * 16], cen[:, 0:C * 16],
                        Alu.mult)
                    varr = lpool.tile([128, LNP * 2], f32, tag="varr")
                    nc.vector.tensor_reduce(
                        varr[:, 0:C * 2],
                        vap(sq2[:], 0, [[LNP * 16, 128], [8, C * 2], [1, 8]]),
                        mybir.AxisListType.X, Alu.add)
                    # rstd = 1/sqrt(var_raw/8 + eps)
                    nc.vector.tensor_scalar(
                        varr[:, 0:C * 2], varr[:, 0:C * 2],
                        0.125, LN_EPS, Alu.mult, Alu.add)
                    nc.scalar.activation(varr[:, 0:C * 2], varr[:, 0:C * 2],
                                         Act.Sqrt)
                    nc.vector.reciprocal(varr[:, 0:C * 2], varr[:, 0:C * 2])
                    # ln = centered * rstd * w + b ; relu
                    lnv = lpool.tile([128, LNP * 16], f32, tag="lnv")
                    lnva = lnv[:]
                    nc.vector.tensor_tensor(
                        lnv[:, 0:C * 16], cen[:, 0:C * 16],
                        vap(varr[:], 0, [[LNP * 2, 128], [1, C * 2], [0, 8]]),
                        Alu.mult)
                    nc.vector.tensor_tensor(
                        lnv[:, 0:C * 16], lnv[:, 0:C * 16],
                        vap(lnw_t[:], 0, [[16, 128], [0, C], [1, 16]]),
                        Alu.mult)
                    nc.vector.tensor_tensor(
                        lnv[:, 0:C * 16], lnv[:, 0:C * 16],
                        vap(lnb_t[:], 0, [[16, 128], [0, C], [1, 16]]),
                        Alu.add)
                    nc.scalar.activation(lnv[:, 0:C * 16], lnv[:, 0:C * 16],
                                         Act.Relu)
                    # rat = ln / (norms + eq_eps)
                    nc.vector.tensor_scalar(
                        norms[:, 0:C * 16], norms[:, 0:C * 16],
                        EQ_EPS, None, Alu.add)
                    nc.vector.reciprocal(norms[:, 0:C * 16],
                                         norms[:, 0:C * 16])
                    nc.vector.tensor_tensor(
                        lnv[:, 0:C * 16], lnv[:, 0:C * 16],
                        norms[:, 0:C * 16], Alu.mult)
                    # f = feat * rat[(m, ix1[d])]
                    fo = lpool.tile([128, LNP * 32], f32, tag="fo")
                    foa = fo[:]
                    nc.vector.tensor_tensor(
                        vap(foa, 0, [[LNP * 32, 128], [32, C], [4, 8]]),
                        vap(fa, 0, [[LNP * 32, 128], [32, C], [4, 8]]),
                        vap(lnva, 0, [[LNP * 16, 128], [16, C], [2, 8]]),
                        Alu.mult)
                    nc.vector.tensor_tensor(
                        vap(foa, 1, [[LNP * 32, 128], [32, C], [4, 8], [1, 3]]),
                        vap(fa, 1, [[LNP * 32, 128], [32, C], [4, 8], [1, 3]]),
                        vap(lnva, 1,
                            [[LNP * 16, 128], [16, C], [2, 8], [0, 3]]),
                        Alu.mult)
                    # write back to f table in DRAM
                    nc.sync.dma_start(
                        out=vap(f_dram[:], p * LNP * 128 * 32,
                                [[32, 128], [4096, C], [1, 32]]),
                        in_=fo[:, 0:W])

            # ============ Phase 2: edges ================================
            bpool = ctx.enter_context(tc.tile_pool(name="edges", bufs=3))
            tpool = ctx.enter_context(tc.tile_pool(name="work", bufs=2))
            ppool = ctx.enter_context(
                tc.tile_pool(name="psA", bufs=2, space="PSUM"))
            hpool = ctx.enter_context(
                tc.tile_pool(name="psH", bufs=2, space="PSUM"))
            for b in range(NBk):
                for i in range(KB):
                    c = b * KB + i
                    e0 = c * 128
                    # ---- loads
                    basis = bpool.tile([128, 64], f32, tag="basis")
                    nc.sync.dma_start(
                        out=basis[:],
                        in_=vap(basis_d.ap(), e0 * 64, [[64, 128], [1, 64]]))
                    eft = bpool.tile([32, 128], f32, tag="eft")
                    nc.sync.dma_start(
                        out=eft[:],
                        in_=vap(eft_d.ap(), e0, [[E_PAD, 32], [1, 128]]))
                    slidx = bpool.tile([128, 1], i32, tag="slidx")
                    nc.sync.dma_start(
                        out=slidx[:],
                        in_=vap(slot_d.ap(), e0, [[1, 128], [1, 1]]))
                    sidx = bpool.tile([128, 1], i32, tag="sidx")
                    nc.sync.dma_start(
                        out=sidx[:],
                        in_=vap(srcidx_d.ap(), e0, [[1, 128], [1, 1]]))
                    fU = bpool.tile([128, 32], f32, tag="fU")
                    nc.gpsimd.indirect_dma_start(
                        out=fU[:], out_offset=None,
                        in_=f_dram[:],
                        in_offset=IndirectOffsetOnAxis(ap=sidx[:, :1], axis=0))

                    # ---- MLP on PE
                    hps = hpool.tile([64, 128], f32, tag="hps")
                    nc.tensor.matmul(out=hps[:], lhsT=w1t_t[:], rhs=eft[:],
                                     start=True, stop=True)
                    h65 = tpool.tile([65, 128], f32, tag="h65")
                    nc.scalar.activation(h65[0:64, :], hps[:], Act.Relu,
                                         bias=b1_t[:, 0:1])
                    nc.gpsimd.memset(h65[64:65, :], 1.0)
                    rw = ppool.tile([128, 768], f32, tag="rw")
                    nc.tensor.matmul(out=rw[:, 0:512], lhsT=h65[:],
                                     rhs=w2b_t[:, 0:512], start=True,
                                     stop=True)
                    nc.tensor.matmul(out=rw[:, 512:768], lhsT=h65[:],
                                     rhs=w2b_t[:, 512:768], start=True,
                                     stop=True)

                    # ---- tmp = fU (x) basis -> tmp2 in (d2, j) layout
                    cdt = mybir.dt.bfloat16 if OPT.get("bf16") else f32
                    e_tmp = nc.gpsimd if OPT.get("split") == 2 else nc.vector
                    e_v = nc.gpsimd if OPT.get("split") else nc.vector
                    pt = tpool.tile([128, 512], f32, tag="pt")
                    pta = pt[:]
                    e_tmp.tensor_tensor(
                        pt[:],
                        vap(fU[:], 0, [[32, 128], [4, 8], [0, 16], [1, 4]]),
                        vap(basis[:], 0,
                            [[64, 128], [0, 8], [1, 16], [16, 4]]),
                        Alu.mult)
                    t1 = tpool.tile([128, 256], f32, tag="t1")
                    e_tmp.tensor_tensor(
                        vap(t1[:], 0, [[256, 128], [2, 128], [1, 2]]),
                        vap(pta, 0, [[512, 128], [4, 128], [1, 2]]),
                        vap(pta, 2, [[512, 128], [4, 128], [1, 2]]),
                        Alu.add)
                    tmp2 = tpool.tile([128, 128], cdt, tag="tmp")
                    tmp2a = tmp2[:]
                    e_tmp.tensor_tensor(
                        vap(tmp2a, 0, [[128, 128], [4, 8], [1, 4], [32, 4]]),
                        vap(t1[:], 0, [[256, 128], [32, 8], [8, 4], [2, 4]]),
                        vap(t1[:], 1, [[256, 128], [32, 8], [8, 4], [2, 4]]),
                        Alu.add)

                    # ---- conv products + j-tree, split into kq and v parts
                    rwsrc = rw[:]
                    if OPT.get("bf16"):
                        rwb = tpool.tile([128, 768], cdt, tag="rwb")
                        nc.vector.tensor_copy(rwb[:], rw[:])
                        rwsrc = rwb[:]

                    def conv_tree(eng, i0, ni, tag, final_out=None,
                                  final_pitch=None):
                        pitch = ni * 128
                        pcT = tpool.tile([128, pitch], cdt, tag=f"pc{tag}")
                        eng.tensor_tensor(
                            pcT[:],
                            vap(rwsrc, i0 * 32,
                                [[768, 128], [32, ni], [0, 4], [1, 32]]),
                            vap(tmp2a, 0,
                                [[128, 128], [0, ni], [32, 4], [1, 32]]),
                            Alu.mult)
                        cur, wj = pcT[:], 32
                        lvl = 0
                        while wj > 1:
                            pin = ni * 4 * wj
                            wj2 = wj // 2
                            if wj2 == 1 and final_out is not None:
                                oap = final_out
                                opitch = final_pitch
                            else:
                                nxt = tpool.tile([128, ni * 4 * wj2], cdt,
                                                 tag=f"ct{tag}{lvl}")
                                oap = vap(nxt[:], 0, [[ni * 4 * wj2, 128],
                                                      [4 * wj2, ni], [wj2, 4],
                                                      [1, wj2]])
                                opitch = None
                            eng.tensor_tensor(
                                oap,
                                vap(cur, 0, [[pin, 128], [4 * wj, ni],
                                             [wj, 4], [1, wj2]]),
                                vap(cur, wj2, [[pin, 128], [4 * wj, ni],
                                               [wj, 4], [1, wj2]]),
                                Alu.add)
                            if wj2 == 1 and final_out is not None:
                                return None
                            cur, wj, lvl = nxt[:], wj2, lvl + 1
                        return cur

                    pay = tpool.tile([128, 36], f32, tag="pay")
                    paya = pay[:]
                    ckq = conv_tree(nc.vector, 0, 16, "kq")   # (128, 64)
                    conv_tree(e_v, 16, 8, "v",
                              final_out=vap(paya, 4,
                                            [[36, 128], [4, 8], [1, 4], [1, 1]]))

                    # ---- scores -> pay[:, 0:4] ; scatter to node slots
                    ps = tpool.tile([128, 32], f32, tag="ps")
                    nc.vector.tensor_tensor(
                        ps[:],
                        vap(ckq, 0, [[64, 128], [1, 32]]),
                        vap(ckq, 32, [[64, 128], [1, 32]]),
                        Alu.mult)
                    sc4 = tpool.tile([128, 4], f32, tag="sc4")
                    nc.vector.tensor_reduce(
                        sc4[:],
                        vap(ps[:], 0, [[32, 128], [8, 4], [1, 8]]),
                        mybir.AxisListType.X, Alu.add)
                    nc.vector.tensor_scalar(
                        sc4[:], sc4[:], SCALE, None, Alu.mult)
                    nc.vector.scalar_tensor_tensor(
                        pay[:, 0:4], sc4[:], 0.2, sc4[:], Alu.mult, Alu.max)
                    nc.gpsimd.indirect_dma_start(
                        out=sc_dram[:], out_offset=IndirectOffsetOnAxis(
                            ap=slidx[:, :1], axis=0),
                        in_=pay[:], in_offset=None)

                # ---- bucket softmax: per-node max, exp, divide ------------
                PB = tpool.tile([128, S * 36], f32, tag="PB")
                PBa = PB[:]
                nc.sync.dma_start(
                    out=PB[:],
                    in_=vap(sc_dram[:], b * RPB * 36,
                            [[S * 36, 128], [1, S * 36]]))
                Mx = tpool.tile([128, 4], f32, tag="Mx")
                nc.vector.tensor_reduce(
                    Mx[:],
                    vap(PBa, 0, [[S * 36, 128], [1, 4], [36, S]]),
                    mybir.AxisListType.X, Alu.max)
                exs = tpool.tile([128, S * 4], f32, tag="exs")
                exsa = exs[:]
                nc.vector.tensor_tensor(
                    vap(exsa, 0, [[S * 4, 128], [4, S], [1, 4]]),
                    vap(PBa, 0, [[S * 36, 128], [36, S], [1, 4]]),
                    vap(Mx[:], 0, [[4, 128], [0, S], [1, 4]]),
                    Alu.subtract)
                nc.scalar.activation(exs[:], exs[:], Act.Exp)
                den = tpool.tile([128, 4], f32, tag="den")
                nc.vector.tensor_reduce(
                    den[:],
                    vap(exsa, 0, [[S * 4, 128], [1, 4], [4, S]]),
                    mybir.AxisListType.X, Alu.add)
                nc.vector.reciprocal(den[:], den[:])
                exv = tpool.tile([128, S * 32], f32, tag="exv")
                exva = exv[:]
                nc.vector.tensor_tensor(
                    vap(exva, 0, [[S * 32, 128], [32, S], [8, 4], [1, 8]]),
                    vap(exsa, 0, [[S * 4, 128], [4, S], [1, 4], [0, 8]]),
                    vap(PBa, 4, [[S * 36, 128], [36, S], [8, 4], [1, 8]]),
                    Alu.mult)
                wcur, wlen = exva, S * 32
                while wlen > 32:
                    half = wlen // 2
                    nxtw = tpool.tile([128, half], f32,
                                      tag=f"sv{half}")
                    nc.vector.tensor_tensor(
                        nxtw[:],
                        vap(wcur, 0, [[wlen if wcur is exva else half * 2,
                                       128], [1, half]]),
                        vap(wcur, half, [[wlen if wcur is exva else half * 2,
                                          128], [1, half]]),
                        Alu.add)
                    wcur, wlen = nxtw[:], half
                outn_a = outn[:]
                nc.vector.tensor_tensor(
                    vap(outn_a, b * 32,
                        [[NBk * 32, 128], [8, 4], [1, 8]]),
                    vap(wcur, 0, [[32, 128], [8, 4], [1, 8]]),
                    vap(den[:], 0, [[4, 128], [1, 4], [0, 8]]),
                    Alu.mult)

            # ============ Phase 3: project, store =======================
            osca = outn[:]
            res = cpool.tile([128, NBk * 32], f32)
            resa = res[:]
            scr = cpool.tile([128, NBk * 32], f32)
            scra = scr[:]
            for mp in range(8):
                tgt = resa if mp == 0 else scra
                nc.vector.tensor_tensor(
                    vap(tgt, 0, [[NBk * 32, 128], [32, NBk], [4, 8], [1, 4]]),
                    vap(osca, mp * 4,
                        [[NBk * 32, 128], [32, NBk], [0, 8], [1, 4]]),
                    vap(proj_t[:], mp * 32,
                        [[256, 128], [0, NBk], [4, 8], [1, 4]]),
                    Alu.mult)
                if mp > 0:
                    nc.vector.tensor_tensor(resa, resa, scra, Alu.add)
            nc.sync.dma_start(
                out=vap(out_d.ap(), 0, [[32, 128], [4096, NBk], [1, 32]]),
                in_=res[:])
    return nc


# ---------------------------------------------------------------------------
# Host-side sharding / prep
# ---------------------------------------------------------------------------
def _prep(inputs, cfg: Cfg = None):
    src = np.asarray(inputs["src"]).astype(np.int64)
    dst = np.asarray(inputs["dst"]).astype(np.int64)
    basis = np.asarray(inputs["basis"], dtype=np.float32).reshape(N_EDGES, 64)
    ef = np.asarray(inputs["edge_feats"], dtype=np.float32)
    feats = np.asarray(inputs["features"], dtype=np.float32)

    nb_g = N_CORES * (cfg.nb if cfg is not None else NB)
    # degree-balanced snake packing of nodes into buckets
    deg = np.bincount(dst, minlength=NODES_PAD)
    order = np.argsort(-deg, kind="stable")
    assign = np.empty(NODES_PAD, dtype=np.int64)
    pos = np.empty(NODES_PAD, dtype=np.int64)
    for r in range(BUCKET_N):
        sl = order[r * nb_g:(r + 1) * nb_g]
        buckets = np.arange(nb_g) if r % 2 == 0 else np.arange(nb_g)[::-1]
        assign[sl] = buckets
        pos[sl] = r
    loads = np.zeros(nb_g, dtype=np.int64)
    np.add.at(loads, assign[dst], 1)
    kb = int(math.ceil(loads.max() / 128.0))
    maxdeg = int(deg.max())
    S = 4
    while S < maxdeg:
        S *= 2
    if cfg is None:
        cfg = Cfg(nb=NB, kb=kb, ln_chunks=NODES_PAD // 128, ln_pass=64,
                  nodes_tbl=NODES_PAD, slots=S)
    assert kb <= cfg.kb, f"kb={kb} exceeds cfg.kb={cfg.kb}"
    assert maxdeg <= cfg.slots, f"maxdeg={maxdeg} > slots={cfg.slots}"

    eb = assign[dst]
    eorder = np.argsort(eb, kind="stable")
    bstart = np.searchsorted(eb[eorder], np.arange(nb_g + 1))

    E_PAD, CH = cfg.e_pad, cfg.ch
    in_maps = []
    for core in range(N_CORES):
        basis_s = np.zeros((E_PAD, 64), np.float32)
        eft_s = np.zeros((32, E_PAD), np.float32)
        srcidx_s = np.zeros((E_PAD, 1), np.int32)
        Sv = cfg.slots
        slot_s = np.full((E_PAD, 1), cfg.nb * 128 * Sv, np.int32)  # dump row
        for lb in range(cfg.nb):
            gb = core * cfg.nb + lb
            eidx = eorder[bstart[gb]:bstart[gb + 1]]
            n = len(eidx)
            assert n <= cfg.kb * 128
            o = lb * cfg.kb * 128
            basis_s[o:o + n] = basis[eidx]
            eft_s[:, o:o + n] = ef[eidx].T
            srcidx_s[o:o + n, 0] = src[eidx]
            p = pos[dst[eidx]]
            o2 = np.argsort(p, kind="stable")
            sp = p[o2]
            if n:
                starts = np.flatnonzero(np.r_[True, sp[1:] != sp[:-1]])
                runlen = np.diff(np.r_[starts, n])
                ks = np.arange(n) - np.repeat(starts, runlen)
                ke = np.empty(n, np.int64)
                ke[o2] = ks
                slot_s[o:o + n, 0] = (lb * 128 + p) * Sv + ke
        in_maps.append({
            "basis_s": basis_s,
            "eft_s": eft_s,
            "srcidx_s": srcidx_s,
            "slot_s": slot_s,
        })

    # replicated constants
    featp = np.zeros((NODES_PAD, 32), np.float32)
    featp[:N_NODES] = feats.reshape(N_NODES, 32)
    featcm = np.ascontiguousarray(
        featp.reshape(cfg.ln_chunks, 128, 32).transpose(1, 0, 2)
        .reshape(128, cfg.ln_chunks * 32))
    mr = np.arange(16)
    lnw_flat = np.asarray(inputs["ln_w"], np.float32)[(mr % 8)]
    lnb_flat = np.asarray(inputs["ln_b"], np.float32)[(mr % 8)]
    lnw = np.broadcast_to(lnw_flat, (128, 16)).copy()
    lnb = np.broadcast_to(lnb_flat, (128, 16)).copy()
    w1 = np.asarray(inputs["w1"], np.float32)
    b1 = np.asarray(inputs["b1"], np.float32).reshape(64, 1)
    w2 = np.asarray(inputs["w2"], np.float32)
    b2 = np.asarray(inputs["b2"], np.float32)
    w2b = np.concatenate([w2.T, b2[None, :]], axis=0).astype(np.float32)
    projw = np.asarray(inputs["proj_w"], np.float32)
    ptbl_flat = np.zeros((256,), np.float32)
    for mpi in range(8):
        for m in range(8):
            for d in range(4):
                ptbl_flat[mpi * 32 + m * 4 + d] = projw[IX2[d] * 8 + m, mpi]
    ptbl = np.broadcast_to(ptbl_flat, (128, 256)).copy()

    for im in in_maps:
        im.update({
            "featcm_s": featcm,
            "lnw_s": lnw,
            "lnb_s": lnb,
            "w1t_s": np.ascontiguousarray(w1.T),
            "b1_s": b1,
            "w2b_s": w2b,
            "proj_s": ptbl,
        })

    meta = {"assign": assign, "pos": pos}
    return in_maps, meta, cfg


def _unshard(results, meta):
    out_cat = np.concatenate([r["out_s"] for r in results], axis=0)
    assign, pos = meta["assign"], meta["pos"]
    rows = assign[:N_NODES] * 128 + pos[:N_NODES]
    return out_cat[rows].reshape(N_NODES, M2, D2)


def _run(inputs, trace=False):
    _apply_patches()
    import concourse.bass as bass
    from concourse.bass_utils import run_bass_kernel_spmd

    in_maps, meta, cfg = _prep(inputs)
    nc = bass.Bass("TRN2", target_bir_lowering=False)
    build_kernel(nc, cfg)
    r = run_bass_kernel_spmd(nc, in_maps, core_ids=list(range(N_CORES)),
                             trace=trace)
    out = _unshard(r.results, meta)
    return out, r


def kernel(**inputs) -> np.ndarray:
    out, _ = _run(inputs, trace=False)
    return out.astype(np.float32)
